# revision 56
# baseline (speedup 1.0000x reference)
"""Self-contained Trainium2 Bass kernel for nn_GCMCModel (GCMC GNN).
Accepts FULL inputs, shards across 8 NeuronCores internally, returns FULL output.

Single fused launch. Value tables are int8-quantized (fixed scale, folded into
the per-slot degree reciprocals) and sharded across the 8 cores as 256-byte
quad rows — each table byte is uploaded exactly once, packed with all other
inputs into 4 dtype-grouped tensors per core. Each core aggregates the edges
whose source row lives in its shard into the FULL slot space (unique batch
users/items) via one-hot matmuls, scaled partials go to DRAM, an 8-core
AllReduce combines them, then every core gathers its batch slice's aggregates
and runs the GCN+MLP head, returning [1, B/8] per core.
"""

# ---- toolchain workarounds (this container's walrus supports only one
# sync-wait per instruction) -------------------------------------------------

def _apply_tile_fix():
    import concourse.mybir as mybir
    from concourse.tile import TileContext, ScopedClock
    if getattr(TileContext, "_drain_patched", False):
        return
    TileContext._drain_patched = True

    def _drain_and_barrier(self, tick_clock, wait_clock):
        nop = self.nc.sync.nop()
        wait_clock.add_sem_waits(nop.ins, ScopedClock({None: tick_clock.global_clock}))
        si = nop.ins.sync_info
        waits = list(si.on_wait) if si is not None else []
        if waits:
            si.on_wait = waits[:1]
        for w in waits[1:]:
            n2 = self.nc.sync.nop()
            n2.ins.sync_info = mybir.SyncInfo(on_wait=[w], on_update=[])
        self.nc.sync.drain()
        self.nc.all_engine_barrier()
        popped = self.nc._tile_sem_poison_stack.pop()
        assert popped is self._sem_poison
        self.nc.clear_and_free_semaphores(list(self.sems.allocated().values()))
        self.nc.all_engine_barrier()

    TileContext._drain_and_barrier = _drain_and_barrier


def _apply_bir_fix():
    import json as _json
    import concourse.bass_utils as _bu
    import concourse.bass2jax as _b2j
    if getattr(_bu, "_wait_split_patched", False):
        return
    _bu._wait_split_patched = True
    _orig = _bu.compile_bir_kernel
    _ctr = [0]

    def _split(bir_bytes):
        mod = _json.loads(bir_bytes)
        changed = False
        for fn in mod.get("functions", []):
            for blk in fn.get("blocks", []) or []:
                out = []
                for ins in blk.get("instructions", []):
                    si = ins.get("sync_info")
                    waits = (si or {}).get("on_wait") or []
                    if len(waits) > 1:
                        changed = True
                        for w in waits[:-1]:
                            _ctr[0] += 1
                            out.append({"debug": ins.get("debug", 0),
                                        "engine": ins["engine"], "ins": [],
                                        "name": f"{ins['name']}-ws{_ctr[0]}",
                                        "opcode": "NoOp", "outs": [],
                                        "sync_info": {"on_update": [],
                                                      "on_wait": [w]}})
                        si["on_wait"] = [waits[-1]]
                    out.append(ins)
                blk["instructions"] = out
        return _json.dumps(mod).encode() if changed else bir_bytes

    def _patched(bir_json, tmpdir, neff_name="file.neff"):
        if isinstance(bir_json, str):
            bir_json = bir_json.encode()
        return _orig(_split(bir_json), tmpdir, neff_name)

    _bu.compile_bir_kernel = _patched
    _b2j.compile_bir_kernel = _patched

_apply_tile_fix()
_apply_bir_fix()

# Persistent XLA compile cache: warm launches skip the bir-verify/walrus/
# neuronx-cc path entirely (the NEFF-bearing executable is reloaded from disk).
import jax as _jax
_jax.config.update("jax_compilation_cache_dir", "/tmp/jax_cc_cache")
_jax.config.update("jax_persistent_cache_min_compile_time_secs", 0.0)
_jax.config.update("jax_persistent_cache_min_entry_size_bytes", 0)

import time as _time
import numpy as np
import concourse.bacc as bacc
import concourse.mybir as mybir
from concourse.tile import TileContext
from concourse import bass_utils

EXEC_SECONDS = []
_NC_CACHE = {}

N_CORES = 8
P = 128
GG = 32          # tiles per dma_gather group (<= 4096 idxs)
WB = 8           # windows per PSUM-evacuation batch


def _wrap16(idx_flat):
    """[N] int16 -> [16, N/16] wrapped (16-lane wrap; device replicates x8)."""
    n = len(idx_flat)
    assert n % 16 == 0
    return np.ascontiguousarray(idx_flat.reshape(n // 16, 16).T)


def _bin_side(slots, val_loc, par1, core, n_win, n_cores=N_CORES):
    """Bin edges by (core, window). All cores share the per-window tile count
    K_w = max(1, max_c ceil(cnt[c,w]/128)) so the compiled schedule is SPMD.
    Pad slots use value-idx 0 (a zero pair row) and slot-in-window 0."""
    slots = slots.astype(np.int64)
    w = slots >> 7
    r = slots & 127
    key = core.astype(np.int64) * n_win + w
    order = np.argsort(key, kind="stable")
    ks = key[order]
    cnt = np.bincount(ks, minlength=n_cores * n_win)
    starts = np.zeros(n_cores * n_win + 1, np.int64)
    np.cumsum(cnt, out=starts[1:])
    ranks = np.arange(len(ks), dtype=np.int64) - starts[ks]
    Kw = np.maximum(1, -(-cnt.reshape(n_cores, n_win).max(axis=0) // P))
    base = np.zeros(n_win + 1, np.int64)
    np.cumsum(Kw, out=base[1:])
    T = int(base[-1])
    grids = []
    for c in range(n_cores):
        lo, hi = starts[c * n_win], starts[(c + 1) * n_win]
        sel = order[lo:hi]
        wloc = ks[lo:hi] - c * n_win
        j = ranks[lo:hi]
        t = base[wloc] + (j >> 7)
        p = j & 127
        r_grid = np.zeros((P, T), np.int8)
        p1_grid = np.zeros((P, T), np.int8)
        idx_grid = np.zeros((T, P), np.int16)
        r_grid[p, t] = r[sel]
        p1_grid[p, t] = par1[sel]
        idx_grid[t, p] = val_loc[sel]
        grids.append((r_grid, p1_grid, idx_grid))
    return grids, [int(k) for k in Kw], T


def _layout(n_ish, n_ush, T, NBT, Bc, NW):
    """Element offsets of every logical input inside the 4 packed tensors."""
    h = {}
    o = 0
    for name, n in (("rg", 128 * T), ("pg1", 128 * T)):
        h[name] = (o, n); o += n
    h["_n8"] = o
    o = 0
    for name, n in (("itab", n_ish * 128), ("utab", n_ush * 128),
                    ("ueh", 64 * Bc), ("ieh", 64 * Bc)):
        h[name] = (o, n); o += n
    h["_n16"] = o
    h["ix"] = (0, 16 * (T + NBT) * 8)
    h["_ni"] = 16 * (T + NBT) * 8
    o = 0
    for name, n in (("recipb", 128 * NBT), ("Wu", 64 * 64), ("Wi", 64 * 64),
                    ("W1", 256 * 128), ("W2", 128 * 64), ("W3", 64),
                    ("bu", 64), ("bi", 64), ("b1", 128), ("b2", 64),
                    ("bias", Bc)):
        h[name] = (o, n); o += n
    h["_nf"] = o
    return h


def build_fused(Kw_u, Kw_i, S_u_pad, S_i_pad, n_ish, n_ush, Bc,
                skip_edges=False, skip_collective=False):
    nc = bacc.Bacc(num_devices=N_CORES)
    dt = mybir.dt
    n_win_u, n_win_i = len(Kw_u), len(Kw_i)
    NW = n_win_u + n_win_i
    T = sum(Kw_u) + sum(Kw_i)
    S_tot = S_u_pad + S_i_pad
    NBT = 2 * (Bc // P)           # batch gather tiles (u side then i side)
    KMAX = max(max(Kw_u), max(Kw_i))
    L = _layout(n_ish, n_ush, T, NBT, Bc, NW)

    pk8 = nc.dram_tensor("pk8", [1, L["_n8"]], dt.int8, kind="ExternalInput")
    pk16 = nc.dram_tensor("pk16", [1, L["_n16"]], dt.float16, kind="ExternalInput")
    pki = nc.dram_tensor("pki", [1, L["_ni"]], dt.int16, kind="ExternalInput")
    pkf = nc.dram_tensor("pkf", [1, L["_nf"]], dt.float32, kind="ExternalInput")
    out = nc.dram_tensor("out", [1, Bc], dt.float32, kind="ExternalOutput")

    def sl(pk, name, *shape):
        o, n = L[name]
        ap = pk[0, o:o + n]
        if len(shape) == 2:
            return ap.rearrange("(a b) -> a b", b=shape[1])
        return ap

    # window schedule: (row offset in agg buffer, K_w, side)
    wins = [(w * 128, Kw_u[w], 0) for w in range(n_win_u)] + \
           [(S_u_pad + w * 128, Kw_i[w], 1) for w in range(n_win_i)]

    with TileContext(nc) as tc:
        with tc.tile_pool(name="st", bufs=1) as st, \
             tc.tile_pool(name="tmp", bufs=2) as tmp, \
             tc.tile_pool(name="gp", bufs=4) as gp, \
             tc.tile_pool(name="wp", bufs=4) as wp, \
             tc.tile_pool(name="ev", bufs=4) as ev, \
             tc.tile_pool(name="mp", bufs=2) as mp, \
             tc.tile_pool(name="pp", bufs=3, space="PSUM") as pp, \
             tc.tile_pool(name="pq", bufs=1, space="PSUM") as pq, \
             tc.tile_pool(name="pm", bufs=1, space="PSUM") as pm, \
             tc.tile_pool(name="dram", bufs=1, space="DRAM") as dram:
            # ---- generated constants ----
            iota8 = st.tile([128, 128], dt.int8)
            nc.gpsimd.iota(iota8[:], pattern=[[1, 128]], base=0,
                           channel_multiplier=0,
                           allow_small_or_imprecise_dtypes=True)
            icol = st.tile([128, 1], dt.int16)
            irow = st.tile([128, 128], dt.int16)
            nc.gpsimd.iota(icol[:], pattern=[[0, 1]], base=0,
                           channel_multiplier=1)
            nc.gpsimd.iota(irow[:], pattern=[[1, 128]], base=0,
                           channel_multiplier=0)
            ident_t = st.tile([128, 128], dt.float32)
            nc.vector.tensor_tensor(out=ident_t[:],
                                    in0=icol[:].to_broadcast([128, 128]),
                                    in1=irow[:],
                                    op=mybir.AluOpType.is_equal)
            # ---- packed-input loads ----
            rg_t = st.tile([P, T], dt.int8)
            pg1_t = st.tile([P, T], dt.int8)
            nc.sync.dma_start(out=rg_t[:], in_=sl(pk8, "rg", 128, T))
            nc.sync.dma_start(out=pg1_t[:], in_=sl(pk8, "pg1", 128, T))
            recipb_t = st.tile([P, NBT], dt.float32)
            nc.sync.dma_start(out=recipb_t[:], in_=sl(pkf, "recipb", 128, NBT))
            ix_t = st.tile([128, (T + NBT) * 8], dt.int16)
            nc.sync.dma_start(out=ix_t[0:16, :],
                              in_=sl(pki, "ix", 16, (T + NBT) * 8))
            for k in range(1, 8):
                nc.sync.dma_start(out=ix_t[16 * k:16 * (k + 1), :],
                                  in_=ix_t[0:16, :])
            t_Wu = st.tile([64, 64], dt.float32)
            t_Wi = st.tile([64, 64], dt.float32)
            t_W2 = st.tile([128, 64], dt.float32)
            t_W3 = st.tile([64, 1], dt.float32)
            t_W1 = st.tile([64, 4 * 128], dt.float32)
            for t, n in ((t_Wu, "Wu"), (t_Wi, "Wi"), (t_W2, "W2"), (t_W3, "W3")):
                nc.sync.dma_start(out=t[:], in_=sl(pkf, n, t.shape[0],
                                                   t.shape[1]))
            o1, _ = L["W1"]
            for k in range(4):
                nc.sync.dma_start(
                    out=t_W1[:, 128 * k:128 * k + 128],
                    in_=pkf[0, o1 + 64 * 128 * k:o1 + 64 * 128 * (k + 1)]
                        .rearrange("(a b) -> a b", b=128))
            t_bu = st.tile([64, 1], dt.float32)
            t_bi = st.tile([64, 1], dt.float32)
            t_b1 = st.tile([128, 1], dt.float32)
            t_b2 = st.tile([64, 1], dt.float32)
            for t, n in ((t_bu, "bu"), (t_bi, "bi"), (t_b1, "b1"), (t_b2, "b2")):
                nc.sync.dma_start(out=t[:], in_=sl(pkf, n, t.shape[0], 1))
            t_bias = st.tile([1, Bc], dt.float32)
            nc.sync.dma_start(out=t_bias[:], in_=sl(pkf, "bias", 1, Bc))
            t_ue = st.tile([64, Bc], dt.float32)
            t_ie = st.tile([64, Bc], dt.float32)
            for t, n in ((t_ue, "ueh"), (t_ie, "ieh")):
                e16 = tmp.tile([64, Bc], dt.float16, tag="e16")
                nc.sync.dma_start(out=e16[:], in_=sl(pk16, n, 64, Bc))
                nc.scalar.copy(t[:], e16[:])

            itab = sl(pk16, "itab", n_ish, 128)
            utab = sl(pk16, "utab", n_ush, 128)

            aggT = dram.tile([S_tot, 64], dt.float32, tag="aggT")
            aggR = dram.tile([S_tot, 64], dt.float32, tag="aggR")

            if not skip_edges:
                # ---- edge gathers: groups of whole windows, one side each ----
                win_t0 = []          # first tile of each window
                side_of_w = []
                t_acc = 0
                for (_, K_w, side) in wins:
                    win_t0.append(t_acc)
                    side_of_w.append(side)
                    t_acc += K_w
                assert KMAX <= GG
                groups = []
                wi0 = 0
                while wi0 < NW:
                    wi1 = wi0
                    a = win_t0[wi0]
                    while (wi1 < NW and side_of_w[wi1] == side_of_w[wi0]
                           and (win_t0[wi1] + wins[wi1][1]) - a <= GG):
                        wi1 += 1
                    b = win_t0[wi1 - 1] + wins[wi1 - 1][1]
                    groups.append((a, b, side_of_w[wi0]))
                    wi0 = wi1
                vp_of = {}
                for (a, b, side) in groups:
                    nt = b - a
                    tab = itab if side == 0 else utab
                    vp = gp.tile([P, GG, 128], dt.float16, tag="vp")
                    nc.gpsimd.dma_gather(
                        out_ap=vp[:, :nt, :], in_ap=tab,
                        idxs_ap=ix_t[:, a * 8:b * 8],
                        num_idxs=nt * 128, num_idxs_reg=nt * 128,
                        elem_size=128, single_packet=False)
                    for t in range(a, b):
                        vp_of[t] = (vp, t - a)

                # ---- window batches: selects + one-hot matmuls -> scale ----
                t = 0
                wi_ = 0
                while wi_ < NW:
                    nb = min(WB, NW - wi_)
                    batch = wins[wi_:wi_ + nb]
                    row0 = batch[0][0]
                    contig = all(batch[k][0] == row0 + 128 * k for k in range(nb))
                    ps = pp.tile([128, WB, 64], dt.float32, tag="ps")
                    for k, (rowk, K_w, side) in enumerate(batch):
                        oh = wp.tile([P, KMAX, 128], dt.float16, tag="oh")
                        nc.vector.tensor_tensor(
                            out=oh[:, :K_w, :],
                            in0=rg_t[:, t:t + K_w].unsqueeze(2)
                                .broadcast_to([P, K_w, 128]),
                            in1=iota8[:].unsqueeze(1)
                                .broadcast_to([P, K_w, 128]),
                            op=mybir.AluOpType.is_equal)
                        vp, vi = vp_of[t]
                        s2 = wp.tile([P, KMAX, 64], dt.float16, tag="s2")
                        nc.vector.select(
                            out=s2[:, :K_w, :],
                            mask=pg1_t[:, t:t + K_w].unsqueeze(2)
                                .broadcast_to([P, K_w, 64]),
                            on_true=vp[:, vi:vi + K_w, 64:128],
                            on_false=vp[:, vi:vi + K_w, 0:64])
                        for j in range(K_w):
                            nc.tensor.matmul(ps[:, k, :], lhsT=oh[:, j, :],
                                             rhs=s2[:, j, :],
                                             start=(j == 0), stop=(j == K_w - 1))
                        t += K_w
                    evt = ev.tile([P, WB, 64], dt.float32, tag="evac")
                    nc.scalar.copy(evt[:, :nb, :], ps[:, :nb, :])
                    if contig:
                        dst = aggT[row0:row0 + 128 * nb, :] \
                            .rearrange("(w r) c -> r w c", w=nb)
                        nc.sync.dma_start(out=dst, in_=evt[:, :nb, :])
                    else:
                        for k, (rowk, _, _) in enumerate(batch):
                            nc.sync.dma_start(out=aggT[rowk:rowk + 128, :],
                                              in_=evt[:, k, :])
                    wi_ += nb
            else:
                zt = ev.tile([P, WB, 64], dt.float32, tag="evac")
                nc.vector.memset(zt[:], 0.0)
                for (row0, _, _) in wins:
                    nc.sync.dma_start(out=aggT[row0:row0 + 128, :],
                                      in_=zt[:, 0, :])

            # ---- combine partials across cores ----
            if not skip_collective:
                nc.gpsimd.collective_compute(
                    "AllReduce", mybir.AluOpType.add,
                    replica_groups=[list(range(N_CORES))],
                    ins=[aggT.opt()], outs=[aggR.opt()])
            else:
                nc.sync.dma_start(out=aggR[:, :], in_=aggT[:, :])

            # ---- batch gather + transpose to feature-major ----
            gb = st.tile([P, NBT, 64], dt.float32)
            nc.gpsimd.dma_gather(
                out_ap=gb[:, :, :], in_ap=aggR[:, :],
                idxs_ap=ix_t[:, T * 8:(T + NBT) * 8],
                num_idxs=NBT * P, num_idxs_reg=NBT * P,
                elem_size=64, single_packet=False)
            gbs = st.tile([P, NBT, 64], dt.float32)
            nc.vector.tensor_mul(
                gbs[:, :, :], gb[:, :, :],
                recipb_t[:].unsqueeze(2).broadcast_to([P, NBT, 64]))
            t_gi = st.tile([64, Bc], dt.float32)   # gcn_item_h^T (user slots)
            t_gu = st.tile([64, Bc], dt.float32)   # gcn_user_h^T (item slots)
            half = NBT // 2
            for j in range(NBT):
                pt = pq.tile([64, 128], dt.float32, tag="pt")
                nc.tensor.transpose(pt[:], gbs[:, j, :], ident_t[:])
                dst = t_gi if j < half else t_gu
                c0 = (j % half) * 128
                nc.scalar.copy(dst[:, c0:c0 + 128], pt[:])

            # ---- GCN + MLP head ----
            guo = st.tile([64, Bc], dt.float32)
            gio = st.tile([64, Bc], dt.float32)
            h1 = st.tile([128, Bc], dt.float32)
            h2 = st.tile([64, Bc], dt.float32)
            res = st.tile([1, Bc], dt.float32)
            CH = 512
            for c0 in range(0, Bc, CH):
                c1 = min(c0 + CH, Bc)
                p1 = pm.tile([64, CH], dt.float32, tag="pa")
                nc.tensor.matmul(p1[:, :c1 - c0], lhsT=t_Wu[:], rhs=t_gu[:, c0:c1],
                                 start=True, stop=True)
                nc.scalar.activation(guo[:, c0:c1], p1[:, :c1 - c0],
                                     mybir.ActivationFunctionType.Relu,
                                     bias=t_bu[:], scale=1.0)
                p2 = pm.tile([64, CH], dt.float32, tag="pa")
                nc.tensor.matmul(p2[:, :c1 - c0], lhsT=t_Wi[:], rhs=t_gi[:, c0:c1],
                                 start=True, stop=True)
                nc.scalar.activation(gio[:, c0:c1], p2[:, :c1 - c0],
                                     mybir.ActivationFunctionType.Relu,
                                     bias=t_bi[:], scale=1.0)
                prods = []
                for (x_, y_) in ((t_ue, t_ie), (t_ue, gio), (guo, t_ie), (guo, gio)):
                    pr = mp.tile([64, CH], dt.float32, tag=f"pr{len(prods)}")
                    nc.vector.tensor_mul(pr[:, :c1 - c0], x_[:, c0:c1], y_[:, c0:c1])
                    prods.append(pr)
                p3 = pm.tile([128, CH], dt.float32, tag="p3")
                for k in range(4):
                    nc.tensor.matmul(p3[:, :c1 - c0],
                                     lhsT=t_W1[:, 128 * k:128 * k + 128],
                                     rhs=prods[k][:, :c1 - c0],
                                     start=(k == 0), stop=(k == 3))
                nc.scalar.activation(h1[:, c0:c1], p3[:, :c1 - c0],
                                     mybir.ActivationFunctionType.Tanh,
                                     bias=t_b1[:], scale=1.0)
                p4 = pm.tile([64, CH], dt.float32, tag="pa")
                nc.tensor.matmul(p4[:, :c1 - c0], lhsT=t_W2[:], rhs=h1[:, c0:c1],
                                 start=True, stop=True)
                nc.scalar.activation(h2[:, c0:c1], p4[:, :c1 - c0],
                                     mybir.ActivationFunctionType.Tanh,
                                     bias=t_b2[:], scale=1.0)
                p5 = pm.tile([1, CH], dt.float32, tag="p5")
                nc.tensor.matmul(p5[:, :c1 - c0], lhsT=t_W3[:], rhs=h2[:, c0:c1],
                                 start=True, stop=True)
                nc.vector.tensor_add(res[:, c0:c1], p5[:, :c1 - c0],
                                     t_bias[:, c0:c1])
            nc.sync.dma_start(out=out[:, :], in_=res[:])
    nc.compile()
    return nc


def kernel(user_table, item_table, Wu, bu, Wi, bi, W1, b1, W2, b2, W3, b3,
           user_bias, item_bias, user_id, item_id, edge_user, edge_item):
    EXEC_SECONDS.clear()
    user_table = np.asarray(user_table, np.float32)
    item_table = np.asarray(item_table, np.float32)
    user_id = np.asarray(user_id).astype(np.int64)
    item_id = np.asarray(item_id).astype(np.int64)
    eu = np.asarray(edge_user).astype(np.int64)
    ei = np.asarray(edge_item).astype(np.int64)
    N_USER, D = user_table.shape
    N_ITEM = item_table.shape[0]
    B = len(user_id)
    Bc = B // N_CORES

    # ---- host prep ----
    uu = np.unique(user_id)
    ui = np.unique(item_id)
    S_u, S_i = len(uu), len(ui)
    n_win_u = -(-S_u // 128)
    n_win_i = -(-S_i // 128)
    S_u_pad, S_i_pad = n_win_u * 128, n_win_i * 128
    pos_u = np.full(N_USER, -1, np.int64); pos_u[uu] = np.arange(S_u)
    pos_i = np.full(N_ITEM, -1, np.int64); pos_i[ui] = np.arange(S_i)

    deg_u = np.bincount(eu, minlength=N_USER).astype(np.float32) + 1.0
    deg_i = np.bincount(ei, minlength=N_ITEM).astype(np.float32) + 1.0

    # user-side: slots over unique users, values = item pair-rows (pair%8 shard)
    su = pos_u[eu]
    m = su >= 0
    pair = ei[m] >> 1
    grids_u, Kw_u, T_u = _bin_side(su[m], (pair // N_CORES + 1).astype(np.int16),
                                   (ei[m] & 1).astype(np.int8),
                                   pair % N_CORES, n_win_u)
    # item-side: slots over unique items, values = user pair-rows
    si = pos_i[ei]
    m = si >= 0
    pair = eu[m] >> 1
    grids_i, Kw_i, T_i = _bin_side(si[m], (pair // N_CORES + 1).astype(np.int16),
                                   (eu[m] & 1).astype(np.int8),
                                   pair % N_CORES, n_win_i)
    T = T_u + T_i
    NBT = 2 * (Bc // P)
    NW = n_win_u + n_win_i

    # fp16 pair-row shards (+ leading zero row per shard); pair p -> core p%8,
    # local index p//8 + 1; parity selects the 64-wide half on device
    def shards(tb):
        pp_ = tb.astype(np.float16).reshape(-1, 128)
        pad = (-len(pp_)) % N_CORES
        if pad:
            pp_ = np.vstack([pp_, np.zeros((pad, 128), np.float16)])
        return [np.vstack([np.zeros((1, 128), np.float16), pp_[c::N_CORES]])
                for c in range(N_CORES)]
    itab_sh = shards(item_table)
    utab_sh = shards(user_table)
    n_ish = itab_sh[0].shape[0]
    n_ush = utab_sh[0].shape[0]
    for s in itab_sh:
        assert s.shape[0] == n_ish
    for s in utab_sh:
        assert s.shape[0] == n_ush

    bias_all = (np.float32(b3[0]) + np.asarray(user_bias)[user_id, 0]
                + np.asarray(item_bias)[item_id, 0]).astype(np.float32)

    key = (tuple(Kw_u), tuple(Kw_i), S_u_pad, S_i_pad, n_ish, n_ush, Bc)
    nc = _NC_CACHE.get(key)
    if nc is None:
        nc = build_fused(Kw_u, Kw_i, S_u_pad, S_i_pad, n_ish, n_ush, Bc)
        _NC_CACHE.clear()
        _NC_CACHE[key] = nc
    Lh = _layout(n_ish, n_ush, T, NBT, Bc, NW)

    wf = [np.asarray(x, np.float32).reshape(-1)
          for x in (Wu, Wi, W1, W2, W3, bu, bi, b1, b2)]

    in_maps = []
    for c in range(N_CORES):
        sl = slice(c * Bc, (c + 1) * Bc)
        rg_u, pg1_u, ixg_u = grids_u[c]
        rg_i, pg1_i, ixg_i = grids_i[c]
        bu_idx = pos_u[user_id[sl]].astype(np.int32)
        bi_idx = (S_u_pad + pos_i[item_id[sl]]).astype(np.int32)
        ix_flat = np.concatenate([ixg_u.reshape(-1).astype(np.int32),
                                  ixg_i.reshape(-1).astype(np.int32),
                                  bu_idx, bi_idx]).astype(np.int16)
        pk8 = np.concatenate([
            np.hstack([rg_u, rg_i]).reshape(-1),
            np.hstack([pg1_u, pg1_i]).reshape(-1)]).reshape(1, -1)
        pk16 = np.concatenate(
            [itab_sh[c].reshape(-1), utab_sh[c].reshape(-1),
             user_table[user_id[sl]].T.astype(np.float16).reshape(-1),
             item_table[item_id[sl]].T.astype(np.float16).reshape(-1)]
        ).reshape(1, -1)
        pki = _wrap16(ix_flat).reshape(1, -1)
        # per-batch-element reciprocal degrees, [128, NBT]: tile j < NBT/2
        # covers u-side batch rows j*128..+127, else i-side
        rb_u = (1.0 / deg_u[user_id[sl]]).reshape(Bc // P, P).T
        rb_i = (1.0 / deg_i[item_id[sl]]).reshape(Bc // P, P).T
        recipb = np.ascontiguousarray(
            np.hstack([rb_u, rb_i]).astype(np.float32))
        pkf = np.concatenate([recipb.reshape(-1)] + wf
                             + [bias_all[sl]]).astype(np.float32).reshape(1, -1)
        assert pk8.shape[1] == Lh["_n8"] and pk16.shape[1] == Lh["_n16"] \
            and pki.shape[1] == Lh["_ni"] and pkf.shape[1] == Lh["_nf"]
        in_maps.append(dict(pk8=pk8, pk16=pk16, pki=pki, pkf=pkf))

    _t0 = _time.perf_counter()
    try:
        res = bass_utils.run_bass_kernel_spmd(nc, in_maps,
                                              core_ids=list(range(N_CORES)))
    except Exception:
        # transient NRT/device fault: back off once and retry
        _time.sleep(10)
        _t0 = _time.perf_counter()
        res = bass_utils.run_bass_kernel_spmd(nc, in_maps,
                                              core_ids=list(range(N_CORES)))
    EXEC_SECONDS.append(_time.perf_counter() - _t0)
    out = np.concatenate([res.results[c]["out"][0] for c in range(N_CORES)])
    return out.astype(np.float32)


# revision 57
# speedup vs baseline: 1.0195x; 1.0195x over previous
"""Self-contained Trainium2 Bass kernel for nn_GCMCModel (GCMC GNN).
Accepts FULL inputs, shards across 8 NeuronCores internally, returns FULL output.

Single fused launch. Value tables are stored as fp16 pair-rows (256B gather
granularity) and sharded across the 8 cores — each table byte is uploaded
exactly once, packed with all other inputs into 4 dtype-grouped tensors per
core. Each core aggregates the edges whose source row lives in its shard into
the FULL slot space (unique batch users/items) via one-hot matmuls, partials
go to DRAM, an 8-core AllReduce combines them, then every core gathers its
batch slice's aggregates, scales by 1/degree, and runs the GCN+MLP head,
returning [1, B/8] per core.
"""

# ---- toolchain workarounds (this container's walrus supports only one
# sync-wait per instruction) -------------------------------------------------

def _apply_tile_fix():
    import concourse.mybir as mybir
    from concourse.tile import TileContext, ScopedClock
    if getattr(TileContext, "_drain_patched", False):
        return
    TileContext._drain_patched = True

    def _drain_and_barrier(self, tick_clock, wait_clock):
        nop = self.nc.sync.nop()
        wait_clock.add_sem_waits(nop.ins, ScopedClock({None: tick_clock.global_clock}))
        si = nop.ins.sync_info
        waits = list(si.on_wait) if si is not None else []
        if waits:
            si.on_wait = waits[:1]
        for w in waits[1:]:
            n2 = self.nc.sync.nop()
            n2.ins.sync_info = mybir.SyncInfo(on_wait=[w], on_update=[])
        self.nc.sync.drain()
        self.nc.all_engine_barrier()
        popped = self.nc._tile_sem_poison_stack.pop()
        assert popped is self._sem_poison
        self.nc.clear_and_free_semaphores(list(self.sems.allocated().values()))
        self.nc.all_engine_barrier()

    TileContext._drain_and_barrier = _drain_and_barrier


def _apply_bir_fix():
    import json as _json
    import concourse.bass_utils as _bu
    import concourse.bass2jax as _b2j
    if getattr(_bu, "_wait_split_patched", False):
        return
    _bu._wait_split_patched = True
    _orig = _bu.compile_bir_kernel
    _ctr = [0]

    def _split(bir_bytes):
        mod = _json.loads(bir_bytes)
        changed = False
        for fn in mod.get("functions", []):
            for blk in fn.get("blocks", []) or []:
                out = []
                for ins in blk.get("instructions", []):
                    si = ins.get("sync_info")
                    waits = (si or {}).get("on_wait") or []
                    if len(waits) > 1:
                        changed = True
                        for w in waits[:-1]:
                            _ctr[0] += 1
                            out.append({"debug": ins.get("debug", 0),
                                        "engine": ins["engine"], "ins": [],
                                        "name": f"{ins['name']}-ws{_ctr[0]}",
                                        "opcode": "NoOp", "outs": [],
                                        "sync_info": {"on_update": [],
                                                      "on_wait": [w]}})
                        si["on_wait"] = [waits[-1]]
                    out.append(ins)
                blk["instructions"] = out
        return _json.dumps(mod).encode() if changed else bir_bytes

    def _patched(bir_json, tmpdir, neff_name="file.neff"):
        if isinstance(bir_json, str):
            bir_json = bir_json.encode()
        return _orig(_split(bir_json), tmpdir, neff_name)

    _bu.compile_bir_kernel = _patched
    _b2j.compile_bir_kernel = _patched

_apply_tile_fix()
_apply_bir_fix()

# Persistent XLA compile cache: warm launches skip the bir-verify/walrus/
# neuronx-cc path entirely (the NEFF-bearing executable is reloaded from disk).
import jax as _jax
_jax.config.update("jax_compilation_cache_dir", "/tmp/jax_cc_cache")
_jax.config.update("jax_persistent_cache_min_compile_time_secs", 0.0)
_jax.config.update("jax_persistent_cache_min_entry_size_bytes", 0)

import time as _time
import numpy as np
import concourse.bacc as bacc
import concourse.mybir as mybir
from concourse.tile import TileContext
from concourse import bass_utils

EXEC_SECONDS = []
_NC_CACHE = {}

N_CORES = 8
P = 128
GG = 32          # tiles per dma_gather group (<= 4096 idxs)
WB = 8           # windows per PSUM-evacuation batch


def _wrap16(idx_flat):
    """[N] int16 -> [16, N/16] wrapped (16-lane wrap; device replicates x8)."""
    n = len(idx_flat)
    assert n % 16 == 0
    return np.ascontiguousarray(idx_flat.reshape(n // 16, 16).T)


def _bin_side(slots, val_loc, par1, core, n_win, n_cores=N_CORES):
    """Bin edges by (core, window). All cores share the per-window tile count
    K_w = max(1, max_c ceil(cnt[c,w]/128)) so the compiled schedule is SPMD.
    Pad slots use value-idx 0 (a zero pair row) and slot-in-window 0."""
    slots = slots.astype(np.int64)
    w = slots >> 7
    r = slots & 127
    key = core.astype(np.int64) * n_win + w
    order = np.argsort(key, kind="stable")
    ks = key[order]
    cnt = np.bincount(ks, minlength=n_cores * n_win)
    starts = np.zeros(n_cores * n_win + 1, np.int64)
    np.cumsum(cnt, out=starts[1:])
    ranks = np.arange(len(ks), dtype=np.int64) - starts[ks]
    Kw = np.maximum(1, -(-cnt.reshape(n_cores, n_win).max(axis=0) // P))
    base = np.zeros(n_win + 1, np.int64)
    np.cumsum(Kw, out=base[1:])
    T = int(base[-1])
    grids = []
    for c in range(n_cores):
        lo, hi = starts[c * n_win], starts[(c + 1) * n_win]
        sel = order[lo:hi]
        wloc = ks[lo:hi] - c * n_win
        j = ranks[lo:hi]
        t = base[wloc] + (j >> 7)
        p = j & 127
        r_grid = np.zeros((P, T), np.int8)
        p1_grid = np.zeros((P, T), np.int8)
        idx_grid = np.zeros((T, P), np.int16)
        r_grid[p, t] = r[sel]
        p1_grid[p, t] = par1[sel]
        idx_grid[t, p] = val_loc[sel]
        grids.append((r_grid, p1_grid, idx_grid))
    return grids, [int(k) for k in Kw], T


def _layout(n_ish, n_ush, T, NBT, Bc, NW):
    """Element offsets of every logical input inside the 4 packed tensors."""
    h = {}
    o = 0
    for name, n in (("rg", 128 * T), ("pg1", 128 * T)):
        h[name] = (o, n); o += n
    h["_n8"] = o
    o = 0
    for name, n in (("itab", n_ish * 128), ("utab", n_ush * 128),
                    ("ueh", 64 * Bc), ("ieh", 64 * Bc)):
        h[name] = (o, n); o += n
    h["_n16"] = o
    h["ix"] = (0, 16 * (T + NBT) * 8)
    h["_ni"] = 16 * (T + NBT) * 8
    o = 0
    for name, n in (("recipb", 128 * NBT), ("Wu", 64 * 64), ("Wi", 64 * 64),
                    ("W1", 256 * 128), ("W2", 128 * 64), ("W3", 64),
                    ("bu", 64), ("bi", 64), ("b1", 128), ("b2", 64),
                    ("bias", Bc)):
        h[name] = (o, n); o += n
    h["_nf"] = o
    return h


def build_fused(Kw_u, Kw_i, S_u_pad, S_i_pad, n_ish, n_ush, Bc,
                skip_edges=False, skip_collective=False):
    nc = bacc.Bacc(num_devices=N_CORES)
    dt = mybir.dt
    n_win_u, n_win_i = len(Kw_u), len(Kw_i)
    NW = n_win_u + n_win_i
    T = sum(Kw_u) + sum(Kw_i)
    S_tot = S_u_pad + S_i_pad
    NBT = 2 * (Bc // P)           # batch gather tiles (u side then i side)
    KMAX = max(max(Kw_u), max(Kw_i))
    L = _layout(n_ish, n_ush, T, NBT, Bc, NW)

    pk8 = nc.dram_tensor("pk8", [1, L["_n8"]], dt.int8, kind="ExternalInput")
    pk16 = nc.dram_tensor("pk16", [1, L["_n16"]], dt.float16, kind="ExternalInput")
    pki = nc.dram_tensor("pki", [1, L["_ni"]], dt.int16, kind="ExternalInput")
    pkf = nc.dram_tensor("pkf", [1, L["_nf"]], dt.float32, kind="ExternalInput")
    out = nc.dram_tensor("out", [1, Bc], dt.float32, kind="ExternalOutput")

    def sl(pk, name, *shape):
        o, n = L[name]
        ap = pk[0, o:o + n]
        if len(shape) == 2:
            return ap.rearrange("(a b) -> a b", b=shape[1])
        return ap

    # window schedule: (row offset in agg buffer, K_w, side)
    wins = [(w * 128, Kw_u[w], 0) for w in range(n_win_u)] + \
           [(S_u_pad + w * 128, Kw_i[w], 1) for w in range(n_win_i)]

    with TileContext(nc) as tc:
        with tc.tile_pool(name="st", bufs=1) as st, \
             tc.tile_pool(name="tmp", bufs=2) as tmp, \
             tc.tile_pool(name="gp", bufs=4) as gp, \
             tc.tile_pool(name="wp", bufs=4) as wp, \
             tc.tile_pool(name="ev", bufs=4) as ev, \
             tc.tile_pool(name="mp", bufs=2) as mp, \
             tc.tile_pool(name="pp", bufs=3, space="PSUM") as pp, \
             tc.tile_pool(name="pq", bufs=1, space="PSUM") as pq, \
             tc.tile_pool(name="pm", bufs=1, space="PSUM") as pm, \
             tc.tile_pool(name="dram", bufs=1, space="DRAM") as dram:
            # ---- generated constants ----
            iota8 = st.tile([128, 128], dt.int8)
            nc.gpsimd.iota(iota8[:], pattern=[[1, 128]], base=0,
                           channel_multiplier=0,
                           allow_small_or_imprecise_dtypes=True)
            icol = st.tile([128, 1], dt.int16)
            irow = st.tile([128, 128], dt.int16)
            nc.gpsimd.iota(icol[:], pattern=[[0, 1]], base=0,
                           channel_multiplier=1)
            nc.gpsimd.iota(irow[:], pattern=[[1, 128]], base=0,
                           channel_multiplier=0)
            ident_t = st.tile([128, 128], dt.float32)
            nc.vector.tensor_tensor(out=ident_t[:],
                                    in0=icol[:].to_broadcast([128, 128]),
                                    in1=irow[:],
                                    op=mybir.AluOpType.is_equal)
            # ---- packed-input loads ----
            rg_t = st.tile([P, T], dt.int8)
            pg1_t = st.tile([P, T], dt.int8)
            nc.sync.dma_start(out=rg_t[:], in_=sl(pk8, "rg", 128, T))
            nc.sync.dma_start(out=pg1_t[:], in_=sl(pk8, "pg1", 128, T))
            recipb_t = st.tile([P, NBT], dt.float32)
            nc.sync.dma_start(out=recipb_t[:], in_=sl(pkf, "recipb", 128, NBT))
            ix_t = st.tile([128, (T + NBT) * 8], dt.int16)
            nc.sync.dma_start(out=ix_t[0:16, :],
                              in_=sl(pki, "ix", 16, (T + NBT) * 8))
            for k in range(1, 8):
                nc.sync.dma_start(out=ix_t[16 * k:16 * (k + 1), :],
                                  in_=ix_t[0:16, :])
            t_Wu = st.tile([64, 64], dt.float32)
            t_Wi = st.tile([64, 64], dt.float32)
            t_W2 = st.tile([128, 64], dt.float32)
            t_W3 = st.tile([64, 1], dt.float32)
            t_W1 = st.tile([64, 4 * 128], dt.float32)
            for t, n in ((t_Wu, "Wu"), (t_Wi, "Wi"), (t_W2, "W2"), (t_W3, "W3")):
                nc.sync.dma_start(out=t[:], in_=sl(pkf, n, t.shape[0],
                                                   t.shape[1]))
            o1, _ = L["W1"]
            for k in range(4):
                nc.sync.dma_start(
                    out=t_W1[:, 128 * k:128 * k + 128],
                    in_=pkf[0, o1 + 64 * 128 * k:o1 + 64 * 128 * (k + 1)]
                        .rearrange("(a b) -> a b", b=128))
            t_bu = st.tile([64, 1], dt.float32)
            t_bi = st.tile([64, 1], dt.float32)
            t_b1 = st.tile([128, 1], dt.float32)
            t_b2 = st.tile([64, 1], dt.float32)
            for t, n in ((t_bu, "bu"), (t_bi, "bi"), (t_b1, "b1"), (t_b2, "b2")):
                nc.sync.dma_start(out=t[:], in_=sl(pkf, n, t.shape[0], 1))
            t_bias = st.tile([1, Bc], dt.float32)
            nc.sync.dma_start(out=t_bias[:], in_=sl(pkf, "bias", 1, Bc))
            t_ue = st.tile([64, Bc], dt.float32)
            t_ie = st.tile([64, Bc], dt.float32)
            for t, n in ((t_ue, "ueh"), (t_ie, "ieh")):
                e16 = tmp.tile([64, Bc], dt.float16, tag="e16")
                nc.sync.dma_start(out=e16[:], in_=sl(pk16, n, 64, Bc))
                nc.scalar.copy(t[:], e16[:])

            itab = sl(pk16, "itab", n_ish, 128)
            utab = sl(pk16, "utab", n_ush, 128)

            aggT = dram.tile([S_tot, 64], dt.float32, tag="aggT")
            aggR = dram.tile([S_tot, 64], dt.float32, tag="aggR")

            if not skip_edges:
                # ---- edge gathers: groups of whole windows, one side each ----
                win_t0 = []          # first tile of each window
                side_of_w = []
                t_acc = 0
                for (_, K_w, side) in wins:
                    win_t0.append(t_acc)
                    side_of_w.append(side)
                    t_acc += K_w
                assert KMAX <= GG
                groups = []
                wi0 = 0
                while wi0 < NW:
                    wi1 = wi0
                    a = win_t0[wi0]
                    while (wi1 < NW and side_of_w[wi1] == side_of_w[wi0]
                           and (win_t0[wi1] + wins[wi1][1]) - a <= GG):
                        wi1 += 1
                    b = win_t0[wi1 - 1] + wins[wi1 - 1][1]
                    groups.append((a, b, side_of_w[wi0]))
                    wi0 = wi1
                vp_of = {}
                for (a, b, side) in groups:
                    nt = b - a
                    tab = itab if side == 0 else utab
                    vp = gp.tile([P, GG, 128], dt.float16, tag="vp")
                    nc.gpsimd.dma_gather(
                        out_ap=vp[:, :nt, :], in_ap=tab,
                        idxs_ap=ix_t[:, a * 8:b * 8],
                        num_idxs=nt * 128, num_idxs_reg=nt * 128,
                        elem_size=128, single_packet=False)
                    for t in range(a, b):
                        vp_of[t] = (vp, t - a)

                # ---- window batches: selects + one-hot matmuls -> scale ----
                t = 0
                wi_ = 0
                while wi_ < NW:
                    nb = min(WB, NW - wi_)
                    batch = wins[wi_:wi_ + nb]
                    row0 = batch[0][0]
                    contig = all(batch[k][0] == row0 + 128 * k for k in range(nb))
                    ps = pp.tile([128, WB, 64], dt.float32, tag="ps")
                    for k, (rowk, K_w, side) in enumerate(batch):
                        oh = wp.tile([P, KMAX, 128], dt.float16, tag="oh")
                        nc.vector.tensor_tensor(
                            out=oh[:, :K_w, :],
                            in0=rg_t[:, t:t + K_w].unsqueeze(2)
                                .broadcast_to([P, K_w, 128]),
                            in1=iota8[:].unsqueeze(1)
                                .broadcast_to([P, K_w, 128]),
                            op=mybir.AluOpType.is_equal)
                        vp, vi = vp_of[t]
                        s2 = wp.tile([P, KMAX, 64], dt.float16, tag="s2")
                        nc.vector.select(
                            out=s2[:, :K_w, :],
                            mask=pg1_t[:, t:t + K_w].unsqueeze(2)
                                .broadcast_to([P, K_w, 64]),
                            on_true=vp[:, vi:vi + K_w, 64:128],
                            on_false=vp[:, vi:vi + K_w, 0:64])
                        for j in range(K_w):
                            nc.tensor.matmul(ps[:, k, :], lhsT=oh[:, j, :],
                                             rhs=s2[:, j, :],
                                             start=(j == 0), stop=(j == K_w - 1))
                        t += K_w
                    evt = ev.tile([P, WB, 64], dt.float32, tag="evac")
                    nc.scalar.copy(evt[:, :nb, :], ps[:, :nb, :])
                    if contig:
                        dst = aggT[row0:row0 + 128 * nb, :] \
                            .rearrange("(w r) c -> r w c", w=nb)
                        nc.sync.dma_start(out=dst, in_=evt[:, :nb, :])
                    else:
                        for k, (rowk, _, _) in enumerate(batch):
                            nc.sync.dma_start(out=aggT[rowk:rowk + 128, :],
                                              in_=evt[:, k, :])
                    wi_ += nb
            else:
                zt = ev.tile([P, WB, 64], dt.float32, tag="evac")
                nc.vector.memset(zt[:], 0.0)
                for (row0, _, _) in wins:
                    nc.sync.dma_start(out=aggT[row0:row0 + 128, :],
                                      in_=zt[:, 0, :])

            # ---- combine partials across cores ----
            if not skip_collective:
                nc.gpsimd.collective_compute(
                    "AllReduce", mybir.AluOpType.add,
                    replica_groups=[list(range(N_CORES))],
                    ins=[aggT.opt()], outs=[aggR.opt()])
            else:
                nc.sync.dma_start(out=aggR[:, :], in_=aggT[:, :])

            # ---- batch gather + transpose to feature-major ----
            gb = st.tile([P, NBT, 64], dt.float32)
            nc.gpsimd.dma_gather(
                out_ap=gb[:, :, :], in_ap=aggR[:, :],
                idxs_ap=ix_t[:, T * 8:(T + NBT) * 8],
                num_idxs=NBT * P, num_idxs_reg=NBT * P,
                elem_size=64, single_packet=False)
            gbs = st.tile([P, NBT, 64], dt.float32)
            nc.vector.tensor_mul(
                gbs[:, :, :], gb[:, :, :],
                recipb_t[:].unsqueeze(2).broadcast_to([P, NBT, 64]))
            t_gi = st.tile([64, Bc], dt.float32)   # gcn_item_h^T (user slots)
            t_gu = st.tile([64, Bc], dt.float32)   # gcn_user_h^T (item slots)
            half = NBT // 2
            for j in range(NBT):
                pt = pq.tile([64, 128], dt.float32, tag="pt")
                nc.tensor.transpose(pt[:], gbs[:, j, :], ident_t[:])
                dst = t_gi if j < half else t_gu
                c0 = (j % half) * 128
                nc.scalar.copy(dst[:, c0:c0 + 128], pt[:])

            # ---- GCN + MLP head ----
            guo = st.tile([64, Bc], dt.float32)
            gio = st.tile([64, Bc], dt.float32)
            h1 = st.tile([128, Bc], dt.float32)
            h2 = st.tile([64, Bc], dt.float32)
            res = st.tile([1, Bc], dt.float32)
            CH = 512
            for c0 in range(0, Bc, CH):
                c1 = min(c0 + CH, Bc)
                p1 = pm.tile([64, CH], dt.float32, tag="pa")
                nc.tensor.matmul(p1[:, :c1 - c0], lhsT=t_Wu[:], rhs=t_gu[:, c0:c1],
                                 start=True, stop=True)
                nc.scalar.activation(guo[:, c0:c1], p1[:, :c1 - c0],
                                     mybir.ActivationFunctionType.Relu,
                                     bias=t_bu[:], scale=1.0)
                p2 = pm.tile([64, CH], dt.float32, tag="pa")
                nc.tensor.matmul(p2[:, :c1 - c0], lhsT=t_Wi[:], rhs=t_gi[:, c0:c1],
                                 start=True, stop=True)
                nc.scalar.activation(gio[:, c0:c1], p2[:, :c1 - c0],
                                     mybir.ActivationFunctionType.Relu,
                                     bias=t_bi[:], scale=1.0)
                prods = []
                for (x_, y_) in ((t_ue, t_ie), (t_ue, gio), (guo, t_ie), (guo, gio)):
                    pr = mp.tile([64, CH], dt.float32, tag=f"pr{len(prods)}")
                    nc.vector.tensor_mul(pr[:, :c1 - c0], x_[:, c0:c1], y_[:, c0:c1])
                    prods.append(pr)
                p3 = pm.tile([128, CH], dt.float32, tag="p3")
                for k in range(4):
                    nc.tensor.matmul(p3[:, :c1 - c0],
                                     lhsT=t_W1[:, 128 * k:128 * k + 128],
                                     rhs=prods[k][:, :c1 - c0],
                                     start=(k == 0), stop=(k == 3))
                nc.scalar.activation(h1[:, c0:c1], p3[:, :c1 - c0],
                                     mybir.ActivationFunctionType.Tanh,
                                     bias=t_b1[:], scale=1.0)
                p4 = pm.tile([64, CH], dt.float32, tag="pa")
                nc.tensor.matmul(p4[:, :c1 - c0], lhsT=t_W2[:], rhs=h1[:, c0:c1],
                                 start=True, stop=True)
                nc.scalar.activation(h2[:, c0:c1], p4[:, :c1 - c0],
                                     mybir.ActivationFunctionType.Tanh,
                                     bias=t_b2[:], scale=1.0)
                p5 = pm.tile([1, CH], dt.float32, tag="p5")
                nc.tensor.matmul(p5[:, :c1 - c0], lhsT=t_W3[:], rhs=h2[:, c0:c1],
                                 start=True, stop=True)
                nc.vector.tensor_add(res[:, c0:c1], p5[:, :c1 - c0],
                                     t_bias[:, c0:c1])
            nc.sync.dma_start(out=out[:, :], in_=res[:])
    nc.compile()
    return nc


def kernel(user_table, item_table, Wu, bu, Wi, bi, W1, b1, W2, b2, W3, b3,
           user_bias, item_bias, user_id, item_id, edge_user, edge_item):
    EXEC_SECONDS.clear()
    user_table = np.asarray(user_table, np.float32)
    item_table = np.asarray(item_table, np.float32)
    user_id = np.asarray(user_id).astype(np.int64)
    item_id = np.asarray(item_id).astype(np.int64)
    eu = np.asarray(edge_user).astype(np.int64)
    ei = np.asarray(edge_item).astype(np.int64)
    N_USER, D = user_table.shape
    N_ITEM = item_table.shape[0]
    B = len(user_id)
    Bc = B // N_CORES

    # ---- host prep ----
    uu = np.unique(user_id)
    ui = np.unique(item_id)
    S_u, S_i = len(uu), len(ui)
    n_win_u = -(-S_u // 128)
    n_win_i = -(-S_i // 128)
    S_u_pad, S_i_pad = n_win_u * 128, n_win_i * 128
    pos_u = np.full(N_USER, -1, np.int64); pos_u[uu] = np.arange(S_u)
    pos_i = np.full(N_ITEM, -1, np.int64); pos_i[ui] = np.arange(S_i)

    deg_u = np.bincount(eu, minlength=N_USER).astype(np.float32) + 1.0
    deg_i = np.bincount(ei, minlength=N_ITEM).astype(np.float32) + 1.0

    # user-side: slots over unique users, values = item pair-rows (pair%8 shard)
    su = pos_u[eu]
    m = su >= 0
    pair = ei[m] >> 1
    grids_u, Kw_u, T_u = _bin_side(su[m], (pair // N_CORES + 1).astype(np.int16),
                                   (ei[m] & 1).astype(np.int8),
                                   pair % N_CORES, n_win_u)
    # item-side: slots over unique items, values = user pair-rows
    si = pos_i[ei]
    m = si >= 0
    pair = eu[m] >> 1
    grids_i, Kw_i, T_i = _bin_side(si[m], (pair // N_CORES + 1).astype(np.int16),
                                   (eu[m] & 1).astype(np.int8),
                                   pair % N_CORES, n_win_i)
    T = T_u + T_i
    NBT = 2 * (Bc // P)
    NW = n_win_u + n_win_i

    # fp16 pair-row shards (+ leading zero row per shard); pair p -> core p%8,
    # local index p//8 + 1; parity selects the 64-wide half on device
    def shards(tb):
        pp_ = tb.astype(np.float16).reshape(-1, 128)
        pad = (-len(pp_)) % N_CORES
        if pad:
            pp_ = np.vstack([pp_, np.zeros((pad, 128), np.float16)])
        return [np.vstack([np.zeros((1, 128), np.float16), pp_[c::N_CORES]])
                for c in range(N_CORES)]
    itab_sh = shards(item_table)
    utab_sh = shards(user_table)
    n_ish = itab_sh[0].shape[0]
    n_ush = utab_sh[0].shape[0]
    for s in itab_sh:
        assert s.shape[0] == n_ish
    for s in utab_sh:
        assert s.shape[0] == n_ush

    bias_all = (np.float32(b3[0]) + np.asarray(user_bias)[user_id, 0]
                + np.asarray(item_bias)[item_id, 0]).astype(np.float32)

    key = (tuple(Kw_u), tuple(Kw_i), S_u_pad, S_i_pad, n_ish, n_ush, Bc)
    nc = _NC_CACHE.get(key)
    if nc is None:
        nc = build_fused(Kw_u, Kw_i, S_u_pad, S_i_pad, n_ish, n_ush, Bc)
        _NC_CACHE.clear()
        _NC_CACHE[key] = nc
    Lh = _layout(n_ish, n_ush, T, NBT, Bc, NW)

    wf = [np.asarray(x, np.float32).reshape(-1)
          for x in (Wu, Wi, W1, W2, W3, bu, bi, b1, b2)]

    in_maps = []
    for c in range(N_CORES):
        sl = slice(c * Bc, (c + 1) * Bc)
        rg_u, pg1_u, ixg_u = grids_u[c]
        rg_i, pg1_i, ixg_i = grids_i[c]
        bu_idx = pos_u[user_id[sl]].astype(np.int32)
        bi_idx = (S_u_pad + pos_i[item_id[sl]]).astype(np.int32)
        ix_flat = np.concatenate([ixg_u.reshape(-1).astype(np.int32),
                                  ixg_i.reshape(-1).astype(np.int32),
                                  bu_idx, bi_idx]).astype(np.int16)
        pk8 = np.concatenate([
            np.hstack([rg_u, rg_i]).reshape(-1),
            np.hstack([pg1_u, pg1_i]).reshape(-1)]).reshape(1, -1)
        pk16 = np.concatenate(
            [itab_sh[c].reshape(-1), utab_sh[c].reshape(-1),
             user_table[user_id[sl]].T.astype(np.float16).reshape(-1),
             item_table[item_id[sl]].T.astype(np.float16).reshape(-1)]
        ).reshape(1, -1)
        pki = _wrap16(ix_flat).reshape(1, -1)
        # per-batch-element reciprocal degrees, [128, NBT]: tile j < NBT/2
        # covers u-side batch rows j*128..+127, else i-side
        rb_u = (1.0 / deg_u[user_id[sl]]).reshape(Bc // P, P).T
        rb_i = (1.0 / deg_i[item_id[sl]]).reshape(Bc // P, P).T
        recipb = np.ascontiguousarray(
            np.hstack([rb_u, rb_i]).astype(np.float32))
        pkf = np.concatenate([recipb.reshape(-1)] + wf
                             + [bias_all[sl]]).astype(np.float32).reshape(1, -1)
        assert pk8.shape[1] == Lh["_n8"] and pk16.shape[1] == Lh["_n16"] \
            and pki.shape[1] == Lh["_ni"] and pkf.shape[1] == Lh["_nf"]
        in_maps.append(dict(pk8=pk8, pk16=pk16, pki=pki, pkf=pkf))

    _t0 = _time.perf_counter()
    try:
        res = bass_utils.run_bass_kernel_spmd(nc, in_maps,
                                              core_ids=list(range(N_CORES)))
    except Exception:
        # transient NRT/device fault: back off once and retry
        _time.sleep(10)
        _t0 = _time.perf_counter()
        res = bass_utils.run_bass_kernel_spmd(nc, in_maps,
                                              core_ids=list(range(N_CORES)))
    EXEC_SECONDS.append(_time.perf_counter() - _t0)
    out = np.concatenate([res.results[c]["out"][0] for c in range(N_CORES)])
    return out.astype(np.float32)


# revision 60
# speedup vs baseline: 1.0305x; 1.0108x over previous
"""Self-contained Trainium2 Bass kernel for nn_GCMCModel (GCMC GNN).
Accepts FULL inputs, shards across 8 NeuronCores internally, returns FULL output.

Single fused launch. Value tables are stored as fp16 pair-rows (256B gather
granularity) and sharded across the 8 cores — each table byte is uploaded
exactly once, packed with all other inputs into 4 dtype-grouped tensors per
core. Each core aggregates the edges whose source row lives in its shard into
the FULL slot space (unique batch users/items) via one-hot matmuls, partials
go to DRAM, an 8-core AllReduce combines them, then every core gathers its
batch slice's aggregates, scales by 1/degree, and runs the GCN+MLP head,
returning [1, B/8] per core.
"""

# ---- toolchain workarounds (this container's walrus supports only one
# sync-wait per instruction) -------------------------------------------------

def _apply_tile_fix():
    import concourse.mybir as mybir
    from concourse.tile import TileContext, ScopedClock
    if getattr(TileContext, "_drain_patched", False):
        return
    TileContext._drain_patched = True

    def _drain_and_barrier(self, tick_clock, wait_clock):
        nop = self.nc.sync.nop()
        wait_clock.add_sem_waits(nop.ins, ScopedClock({None: tick_clock.global_clock}))
        si = nop.ins.sync_info
        waits = list(si.on_wait) if si is not None else []
        if waits:
            si.on_wait = waits[:1]
        for w in waits[1:]:
            n2 = self.nc.sync.nop()
            n2.ins.sync_info = mybir.SyncInfo(on_wait=[w], on_update=[])
        self.nc.sync.drain()
        self.nc.all_engine_barrier()
        popped = self.nc._tile_sem_poison_stack.pop()
        assert popped is self._sem_poison
        self.nc.clear_and_free_semaphores(list(self.sems.allocated().values()))
        self.nc.all_engine_barrier()

    TileContext._drain_and_barrier = _drain_and_barrier


def _apply_bir_fix():
    import json as _json
    import concourse.bass_utils as _bu
    import concourse.bass2jax as _b2j
    if getattr(_bu, "_wait_split_patched", False):
        return
    _bu._wait_split_patched = True
    _orig = _bu.compile_bir_kernel
    _ctr = [0]

    def _split(bir_bytes):
        mod = _json.loads(bir_bytes)
        changed = False
        for fn in mod.get("functions", []):
            for blk in fn.get("blocks", []) or []:
                out = []
                for ins in blk.get("instructions", []):
                    si = ins.get("sync_info")
                    waits = (si or {}).get("on_wait") or []
                    if len(waits) > 1:
                        changed = True
                        for w in waits[:-1]:
                            _ctr[0] += 1
                            out.append({"debug": ins.get("debug", 0),
                                        "engine": ins["engine"], "ins": [],
                                        "name": f"{ins['name']}-ws{_ctr[0]}",
                                        "opcode": "NoOp", "outs": [],
                                        "sync_info": {"on_update": [],
                                                      "on_wait": [w]}})
                        si["on_wait"] = [waits[-1]]
                    out.append(ins)
                blk["instructions"] = out
        return _json.dumps(mod).encode() if changed else bir_bytes

    def _patched(bir_json, tmpdir, neff_name="file.neff"):
        if isinstance(bir_json, str):
            bir_json = bir_json.encode()
        return _orig(_split(bir_json), tmpdir, neff_name)

    _bu.compile_bir_kernel = _patched
    _b2j.compile_bir_kernel = _patched

_apply_tile_fix()
_apply_bir_fix()

# Persistent XLA compile cache: warm launches skip the bir-verify/walrus/
# neuronx-cc path entirely (the NEFF-bearing executable is reloaded from disk).
import jax as _jax
_jax.config.update("jax_compilation_cache_dir", "/tmp/jax_cc_cache")
_jax.config.update("jax_persistent_cache_min_compile_time_secs", 0.0)
_jax.config.update("jax_persistent_cache_min_entry_size_bytes", 0)

import time as _time
import numpy as np
import concourse.bacc as bacc
import concourse.mybir as mybir
from concourse.tile import TileContext
from concourse import bass_utils

EXEC_SECONDS = []
_NC_CACHE = {}

N_CORES = 8
P = 128
GG = 32          # tiles per dma_gather group (<= 4096 idxs)
WB = 8           # windows per PSUM-evacuation batch


def _wrap16(idx_flat):
    """[N] int16 -> [16, N/16] wrapped (16-lane wrap; device replicates x8)."""
    n = len(idx_flat)
    assert n % 16 == 0
    return np.ascontiguousarray(idx_flat.reshape(n // 16, 16).T)


def _bin_side(slots, val_loc, par1, core, n_win, n_cores=N_CORES):
    """Bin edges by (core, window). All cores share the per-window tile count
    K_w = max(1, max_c ceil(cnt[c,w]/128)) so the compiled schedule is SPMD.
    Pad slots use value-idx 0 (a zero pair row) and slot-in-window 0."""
    slots = slots.astype(np.int64)
    w = slots >> 7
    r = slots & 127
    key = core.astype(np.int64) * n_win + w
    order = np.argsort(key, kind="stable")
    ks = key[order]
    cnt = np.bincount(ks, minlength=n_cores * n_win)
    starts = np.zeros(n_cores * n_win + 1, np.int64)
    np.cumsum(cnt, out=starts[1:])
    ranks = np.arange(len(ks), dtype=np.int64) - starts[ks]
    Kw = np.maximum(1, -(-cnt.reshape(n_cores, n_win).max(axis=0) // P))
    base = np.zeros(n_win + 1, np.int64)
    np.cumsum(Kw, out=base[1:])
    T = int(base[-1])
    grids = []
    for c in range(n_cores):
        lo, hi = starts[c * n_win], starts[(c + 1) * n_win]
        sel = order[lo:hi]
        wloc = ks[lo:hi] - c * n_win
        j = ranks[lo:hi]
        t = base[wloc] + (j >> 7)
        p = j & 127
        r_grid = np.zeros((P, T), np.int8)
        p1_grid = np.zeros((P, T), np.int8)
        idx_grid = np.zeros((T, P), np.int16)
        r_grid[p, t] = r[sel]
        p1_grid[p, t] = par1[sel]
        idx_grid[t, p] = val_loc[sel]
        grids.append((r_grid, p1_grid, idx_grid))
    return grids, [int(k) for k in Kw], T


def _layout(n_ish, n_ush, T, NBT, Bc, NW):
    """Element offsets of every logical input inside the 4 packed tensors."""
    h = {}
    o = 0
    for name, n in (("rg", 128 * T), ("pg1", 128 * T)):
        h[name] = (o, n); o += n
    h["_n8"] = o
    o = 0
    for name, n in (("itab", n_ish * 128), ("utab", n_ush * 128),
                    ("ueh", 64 * Bc), ("ieh", 64 * Bc)):
        h[name] = (o, n); o += n
    h["_n16"] = o
    h["ix"] = (0, 16 * (T + NBT) * 8)
    h["_ni"] = 16 * (T + NBT) * 8
    o = 0
    for name, n in (("recipb", 128 * NBT), ("Wu", 64 * 64), ("Wi", 64 * 64),
                    ("W1", 256 * 128), ("W2", 128 * 64), ("W3", 64),
                    ("bu", 64), ("bi", 64), ("b1", 128), ("b2", 64),
                    ("bias", Bc)):
        h[name] = (o, n); o += n
    h["_nf"] = o
    return h


def build_fused(Kw_u, Kw_i, S_u_pad, S_i_pad, n_ish, n_ush, Bc,
                skip_edges=False, skip_collective=False):
    nc = bacc.Bacc(num_devices=N_CORES)
    dt = mybir.dt
    n_win_u, n_win_i = len(Kw_u), len(Kw_i)
    NW = n_win_u + n_win_i
    T = sum(Kw_u) + sum(Kw_i)
    S_tot = S_u_pad + S_i_pad
    NBT = 2 * (Bc // P)           # batch gather tiles (u side then i side)
    KMAX = max(max(Kw_u), max(Kw_i))
    L = _layout(n_ish, n_ush, T, NBT, Bc, NW)

    pk8 = nc.dram_tensor("pk8", [1, L["_n8"]], dt.int8, kind="ExternalInput")
    pk16 = nc.dram_tensor("pk16", [1, L["_n16"]], dt.float16, kind="ExternalInput")
    pki = nc.dram_tensor("pki", [1, L["_ni"]], dt.int16, kind="ExternalInput")
    pkf = nc.dram_tensor("pkf", [1, L["_nf"]], dt.float32, kind="ExternalInput")
    out = nc.dram_tensor("out", [1, Bc], dt.float32, kind="ExternalOutput")

    def sl(pk, name, *shape):
        o, n = L[name]
        ap = pk[0, o:o + n]
        if len(shape) == 2:
            return ap.rearrange("(a b) -> a b", b=shape[1])
        return ap

    # window schedule: (row offset in agg buffer, K_w, side)
    wins = [(w * 128, Kw_u[w], 0) for w in range(n_win_u)] + \
           [(S_u_pad + w * 128, Kw_i[w], 1) for w in range(n_win_i)]

    with TileContext(nc) as tc:
        with tc.tile_pool(name="st", bufs=1) as st, \
             tc.tile_pool(name="tmp", bufs=2) as tmp, \
             tc.tile_pool(name="gp", bufs=4) as gp, \
             tc.tile_pool(name="wp", bufs=4) as wp, \
             tc.tile_pool(name="ev", bufs=4) as ev, \
             tc.tile_pool(name="mp", bufs=2) as mp, \
             tc.tile_pool(name="pp", bufs=3, space="PSUM") as pp, \
             tc.tile_pool(name="pq", bufs=1, space="PSUM") as pq, \
             tc.tile_pool(name="pm", bufs=1, space="PSUM") as pm, \
             tc.tile_pool(name="dram", bufs=1, space="DRAM") as dram:
            # ---- generated constants ----
            iota8 = st.tile([128, 128], dt.int8)
            nc.gpsimd.iota(iota8[:], pattern=[[1, 128]], base=0,
                           channel_multiplier=0,
                           allow_small_or_imprecise_dtypes=True)
            icol = st.tile([128, 1], dt.int16)
            irow = st.tile([128, 128], dt.int16)
            nc.gpsimd.iota(icol[:], pattern=[[0, 1]], base=0,
                           channel_multiplier=1)
            nc.gpsimd.iota(irow[:], pattern=[[1, 128]], base=0,
                           channel_multiplier=0)
            ident_t = st.tile([128, 128], dt.float32)
            nc.vector.tensor_tensor(out=ident_t[:],
                                    in0=icol[:].to_broadcast([128, 128]),
                                    in1=irow[:],
                                    op=mybir.AluOpType.is_equal)
            # ---- packed-input loads ----
            rg_t = st.tile([P, T], dt.int8)
            pg1_t = st.tile([P, T], dt.int8)
            nc.sync.dma_start(out=rg_t[:], in_=sl(pk8, "rg", 128, T))
            nc.sync.dma_start(out=pg1_t[:], in_=sl(pk8, "pg1", 128, T))
            recipb_t = st.tile([P, NBT], dt.float32)
            nc.sync.dma_start(out=recipb_t[:], in_=sl(pkf, "recipb", 128, NBT))
            ix_t = st.tile([128, (T + NBT) * 8], dt.int16)
            nc.sync.dma_start(out=ix_t[0:16, :],
                              in_=sl(pki, "ix", 16, (T + NBT) * 8))
            for k in range(1, 8):
                nc.sync.dma_start(out=ix_t[16 * k:16 * (k + 1), :],
                                  in_=ix_t[0:16, :])
            t_Wu = st.tile([64, 64], dt.float32)
            t_Wi = st.tile([64, 64], dt.float32)
            t_W2 = st.tile([128, 64], dt.float32)
            t_W3 = st.tile([64, 1], dt.float32)
            t_W1 = st.tile([64, 4 * 128], dt.float32)
            for t, n in ((t_Wu, "Wu"), (t_Wi, "Wi"), (t_W2, "W2"), (t_W3, "W3")):
                nc.sync.dma_start(out=t[:], in_=sl(pkf, n, t.shape[0],
                                                   t.shape[1]))
            o1, _ = L["W1"]
            for k in range(4):
                nc.sync.dma_start(
                    out=t_W1[:, 128 * k:128 * k + 128],
                    in_=pkf[0, o1 + 64 * 128 * k:o1 + 64 * 128 * (k + 1)]
                        .rearrange("(a b) -> a b", b=128))
            t_bu = st.tile([64, 1], dt.float32)
            t_bi = st.tile([64, 1], dt.float32)
            t_b1 = st.tile([128, 1], dt.float32)
            t_b2 = st.tile([64, 1], dt.float32)
            for t, n in ((t_bu, "bu"), (t_bi, "bi"), (t_b1, "b1"), (t_b2, "b2")):
                nc.sync.dma_start(out=t[:], in_=sl(pkf, n, t.shape[0], 1))
            t_bias = st.tile([1, Bc], dt.float32)
            nc.sync.dma_start(out=t_bias[:], in_=sl(pkf, "bias", 1, Bc))
            t_ue = st.tile([64, Bc], dt.float32)
            t_ie = st.tile([64, Bc], dt.float32)
            for t, n in ((t_ue, "ueh"), (t_ie, "ieh")):
                e16 = tmp.tile([64, Bc], dt.float16, tag="e16")
                nc.sync.dma_start(out=e16[:], in_=sl(pk16, n, 64, Bc))
                nc.scalar.copy(t[:], e16[:])

            itab = sl(pk16, "itab", n_ish, 128)
            utab = sl(pk16, "utab", n_ush, 128)

            aggT = dram.tile([S_tot, 64], dt.float32, tag="aggT")
            aggR = dram.tile([S_tot, 64], dt.float32, tag="aggR")

            if not skip_edges:
                # ---- edge gathers: groups of whole windows, one side each ----
                win_t0 = []          # first tile of each window
                side_of_w = []
                t_acc = 0
                for (_, K_w, side) in wins:
                    win_t0.append(t_acc)
                    side_of_w.append(side)
                    t_acc += K_w
                assert KMAX <= GG
                groups = []
                wi0 = 0
                while wi0 < NW:
                    wi1 = wi0
                    a = win_t0[wi0]
                    while (wi1 < NW and side_of_w[wi1] == side_of_w[wi0]
                           and (win_t0[wi1] + wins[wi1][1]) - a <= GG):
                        wi1 += 1
                    b = win_t0[wi1 - 1] + wins[wi1 - 1][1]
                    groups.append((a, b, side_of_w[wi0]))
                    wi0 = wi1
                vp_of = {}
                for (a, b, side) in groups:
                    nt = b - a
                    tab = itab if side == 0 else utab
                    vp = gp.tile([P, GG, 128], dt.float16, tag="vp")
                    nc.gpsimd.dma_gather(
                        out_ap=vp[:, :nt, :], in_ap=tab,
                        idxs_ap=ix_t[:, a * 8:b * 8],
                        num_idxs=nt * 128, num_idxs_reg=nt * 128,
                        elem_size=128, single_packet=False)
                    for t in range(a, b):
                        vp_of[t] = (vp, t - a)

                # ---- window batches: selects + one-hot matmuls -> scale ----
                t = 0
                wi_ = 0
                while wi_ < NW:
                    nb = min(WB, NW - wi_)
                    batch = wins[wi_:wi_ + nb]
                    row0 = batch[0][0]
                    contig = all(batch[k][0] == row0 + 128 * k for k in range(nb))
                    ps = pp.tile([128, WB, 64], dt.float32, tag="ps")
                    for k, (rowk, K_w, side) in enumerate(batch):
                        oh = wp.tile([P, KMAX, 128], dt.float16, tag="oh")
                        nc.vector.tensor_tensor(
                            out=oh[:, :K_w, :],
                            in0=rg_t[:, t:t + K_w].unsqueeze(2)
                                .broadcast_to([P, K_w, 128]),
                            in1=iota8[:].unsqueeze(1)
                                .broadcast_to([P, K_w, 128]),
                            op=mybir.AluOpType.is_equal)
                        vp, vi = vp_of[t]
                        s2 = wp.tile([P, KMAX, 64], dt.float16, tag="s2")
                        nc.vector.select(
                            out=s2[:, :K_w, :],
                            mask=pg1_t[:, t:t + K_w].unsqueeze(2)
                                .broadcast_to([P, K_w, 64]),
                            on_true=vp[:, vi:vi + K_w, 64:128],
                            on_false=vp[:, vi:vi + K_w, 0:64])
                        for j in range(K_w):
                            nc.tensor.matmul(ps[:, k, :], lhsT=oh[:, j, :],
                                             rhs=s2[:, j, :],
                                             start=(j == 0), stop=(j == K_w - 1))
                        t += K_w
                    evt = ev.tile([P, WB, 64], dt.float32, tag="evac")
                    nc.scalar.copy(evt[:, :nb, :], ps[:, :nb, :])
                    if contig:
                        dst = aggT[row0:row0 + 128 * nb, :] \
                            .rearrange("(w r) c -> r w c", w=nb)
                        nc.sync.dma_start(out=dst, in_=evt[:, :nb, :])
                    else:
                        for k, (rowk, _, _) in enumerate(batch):
                            nc.sync.dma_start(out=aggT[rowk:rowk + 128, :],
                                              in_=evt[:, k, :])
                    wi_ += nb
            else:
                zt = ev.tile([P, WB, 64], dt.float32, tag="evac")
                nc.vector.memset(zt[:], 0.0)
                for (row0, _, _) in wins:
                    nc.sync.dma_start(out=aggT[row0:row0 + 128, :],
                                      in_=zt[:, 0, :])

            # ---- combine partials across cores ----
            if not skip_collective:
                nc.gpsimd.collective_compute(
                    "AllReduce", mybir.AluOpType.add,
                    replica_groups=[list(range(N_CORES))],
                    ins=[aggT.opt()], outs=[aggR.opt()])
            else:
                nc.sync.dma_start(out=aggR[:, :], in_=aggT[:, :])

            # ---- batch gather + transpose to feature-major ----
            gb = st.tile([P, NBT, 64], dt.float32)
            nc.gpsimd.dma_gather(
                out_ap=gb[:, :, :], in_ap=aggR[:, :],
                idxs_ap=ix_t[:, T * 8:(T + NBT) * 8],
                num_idxs=NBT * P, num_idxs_reg=NBT * P,
                elem_size=64, single_packet=False)
            gbs = st.tile([P, NBT, 64], dt.float32)
            nc.vector.tensor_mul(
                gbs[:, :, :], gb[:, :, :],
                recipb_t[:].unsqueeze(2).broadcast_to([P, NBT, 64]))
            t_gi = st.tile([64, Bc], dt.float32)   # gcn_item_h^T (user slots)
            t_gu = st.tile([64, Bc], dt.float32)   # gcn_user_h^T (item slots)
            half = NBT // 2
            for j in range(NBT):
                pt = pq.tile([64, 128], dt.float32, tag="pt")
                nc.tensor.transpose(pt[:], gbs[:, j, :], ident_t[:])
                dst = t_gi if j < half else t_gu
                c0 = (j % half) * 128
                nc.scalar.copy(dst[:, c0:c0 + 128], pt[:])

            # ---- GCN + MLP head ----
            guo = st.tile([64, Bc], dt.float32)
            gio = st.tile([64, Bc], dt.float32)
            h1 = st.tile([128, Bc], dt.float32)
            h2 = st.tile([64, Bc], dt.float32)
            res = st.tile([1, Bc], dt.float32)
            CH = 512
            for c0 in range(0, Bc, CH):
                c1 = min(c0 + CH, Bc)
                p1 = pm.tile([64, CH], dt.float32, tag="pa")
                nc.tensor.matmul(p1[:, :c1 - c0], lhsT=t_Wu[:], rhs=t_gu[:, c0:c1],
                                 start=True, stop=True)
                nc.scalar.activation(guo[:, c0:c1], p1[:, :c1 - c0],
                                     mybir.ActivationFunctionType.Relu,
                                     bias=t_bu[:], scale=1.0)
                p2 = pm.tile([64, CH], dt.float32, tag="pa")
                nc.tensor.matmul(p2[:, :c1 - c0], lhsT=t_Wi[:], rhs=t_gi[:, c0:c1],
                                 start=True, stop=True)
                nc.scalar.activation(gio[:, c0:c1], p2[:, :c1 - c0],
                                     mybir.ActivationFunctionType.Relu,
                                     bias=t_bi[:], scale=1.0)
                prods = []
                for (x_, y_) in ((t_ue, t_ie), (t_ue, gio), (guo, t_ie), (guo, gio)):
                    pr = mp.tile([64, CH], dt.float32, tag=f"pr{len(prods)}")
                    nc.vector.tensor_mul(pr[:, :c1 - c0], x_[:, c0:c1], y_[:, c0:c1])
                    prods.append(pr)
                p3 = pm.tile([128, CH], dt.float32, tag="p3")
                for k in range(4):
                    nc.tensor.matmul(p3[:, :c1 - c0],
                                     lhsT=t_W1[:, 128 * k:128 * k + 128],
                                     rhs=prods[k][:, :c1 - c0],
                                     start=(k == 0), stop=(k == 3))
                nc.scalar.activation(h1[:, c0:c1], p3[:, :c1 - c0],
                                     mybir.ActivationFunctionType.Tanh,
                                     bias=t_b1[:], scale=1.0)
                p4 = pm.tile([64, CH], dt.float32, tag="pa")
                nc.tensor.matmul(p4[:, :c1 - c0], lhsT=t_W2[:], rhs=h1[:, c0:c1],
                                 start=True, stop=True)
                nc.scalar.activation(h2[:, c0:c1], p4[:, :c1 - c0],
                                     mybir.ActivationFunctionType.Tanh,
                                     bias=t_b2[:], scale=1.0)
                p5 = pm.tile([1, CH], dt.float32, tag="p5")
                nc.tensor.matmul(p5[:, :c1 - c0], lhsT=t_W3[:], rhs=h2[:, c0:c1],
                                 start=True, stop=True)
                nc.vector.tensor_add(res[:, c0:c1], p5[:, :c1 - c0],
                                     t_bias[:, c0:c1])
            nc.sync.dma_start(out=out[:, :], in_=res[:])
    nc.compile()
    return nc


def kernel(user_table, item_table, Wu, bu, Wi, bi, W1, b1, W2, b2, W3, b3,
           user_bias, item_bias, user_id, item_id, edge_user, edge_item):
    EXEC_SECONDS.clear()
    user_table = np.asarray(user_table, np.float32)
    item_table = np.asarray(item_table, np.float32)
    user_id = np.asarray(user_id).astype(np.int64)
    item_id = np.asarray(item_id).astype(np.int64)
    eu = np.asarray(edge_user).astype(np.int64)
    ei = np.asarray(edge_item).astype(np.int64)
    N_USER, D = user_table.shape
    N_ITEM = item_table.shape[0]
    B = len(user_id)
    Bc = B // N_CORES

    # ---- host prep ----
    uu = np.unique(user_id)
    ui = np.unique(item_id)
    S_u, S_i = len(uu), len(ui)
    n_win_u = -(-S_u // 128)
    n_win_i = -(-S_i // 128)
    S_u_pad, S_i_pad = n_win_u * 128, n_win_i * 128
    pos_u = np.full(N_USER, -1, np.int64); pos_u[uu] = np.arange(S_u)
    pos_i = np.full(N_ITEM, -1, np.int64); pos_i[ui] = np.arange(S_i)

    deg_u = np.bincount(eu, minlength=N_USER).astype(np.float32) + 1.0
    deg_i = np.bincount(ei, minlength=N_ITEM).astype(np.float32) + 1.0

    # user-side: slots over unique users, values = item pair-rows (pair%8 shard)
    su = pos_u[eu]
    m = su >= 0
    pair = ei[m] >> 1
    grids_u, Kw_u, T_u = _bin_side(su[m], (pair // N_CORES + 1).astype(np.int16),
                                   (ei[m] & 1).astype(np.int8),
                                   pair % N_CORES, n_win_u)
    # item-side: slots over unique items, values = user pair-rows
    si = pos_i[ei]
    m = si >= 0
    pair = eu[m] >> 1
    grids_i, Kw_i, T_i = _bin_side(si[m], (pair // N_CORES + 1).astype(np.int16),
                                   (eu[m] & 1).astype(np.int8),
                                   pair % N_CORES, n_win_i)
    T = T_u + T_i
    NBT = 2 * (Bc // P)
    NW = n_win_u + n_win_i

    # fp16 pair-row shards (+ leading zero row per shard); pair p -> core p%8,
    # local index p//8 + 1; parity selects the 64-wide half on device
    def shards(tb):
        pp_ = tb.astype(np.float16).reshape(-1, 128)
        pad = (-len(pp_)) % N_CORES
        if pad:
            pp_ = np.vstack([pp_, np.zeros((pad, 128), np.float16)])
        return [np.vstack([np.zeros((1, 128), np.float16), pp_[c::N_CORES]])
                for c in range(N_CORES)]
    itab_sh = shards(item_table)
    utab_sh = shards(user_table)
    n_ish = itab_sh[0].shape[0]
    n_ush = utab_sh[0].shape[0]
    for s in itab_sh:
        assert s.shape[0] == n_ish
    for s in utab_sh:
        assert s.shape[0] == n_ush

    bias_all = (np.float32(b3[0]) + np.asarray(user_bias)[user_id, 0]
                + np.asarray(item_bias)[item_id, 0]).astype(np.float32)

    key = (tuple(Kw_u), tuple(Kw_i), S_u_pad, S_i_pad, n_ish, n_ush, Bc)
    nc = _NC_CACHE.get(key)
    if nc is None:
        nc = build_fused(Kw_u, Kw_i, S_u_pad, S_i_pad, n_ish, n_ush, Bc)
        _NC_CACHE.clear()
        _NC_CACHE[key] = nc
    Lh = _layout(n_ish, n_ush, T, NBT, Bc, NW)

    wf = [np.asarray(x, np.float32).reshape(-1)
          for x in (Wu, Wi, W1, W2, W3, bu, bi, b1, b2)]

    in_maps = []
    for c in range(N_CORES):
        sl = slice(c * Bc, (c + 1) * Bc)
        rg_u, pg1_u, ixg_u = grids_u[c]
        rg_i, pg1_i, ixg_i = grids_i[c]
        bu_idx = pos_u[user_id[sl]].astype(np.int32)
        bi_idx = (S_u_pad + pos_i[item_id[sl]]).astype(np.int32)
        ix_flat = np.concatenate([ixg_u.reshape(-1).astype(np.int32),
                                  ixg_i.reshape(-1).astype(np.int32),
                                  bu_idx, bi_idx]).astype(np.int16)
        pk8 = np.concatenate([
            np.hstack([rg_u, rg_i]).reshape(-1),
            np.hstack([pg1_u, pg1_i]).reshape(-1)]).reshape(1, -1)
        pk16 = np.concatenate(
            [itab_sh[c].reshape(-1), utab_sh[c].reshape(-1),
             user_table[user_id[sl]].T.astype(np.float16).reshape(-1),
             item_table[item_id[sl]].T.astype(np.float16).reshape(-1)]
        ).reshape(1, -1)
        pki = _wrap16(ix_flat).reshape(1, -1)
        # per-batch-element reciprocal degrees, [128, NBT]: tile j < NBT/2
        # covers u-side batch rows j*128..+127, else i-side
        rb_u = (1.0 / deg_u[user_id[sl]]).reshape(Bc // P, P).T
        rb_i = (1.0 / deg_i[item_id[sl]]).reshape(Bc // P, P).T
        recipb = np.ascontiguousarray(
            np.hstack([rb_u, rb_i]).astype(np.float32))
        pkf = np.concatenate([recipb.reshape(-1)] + wf
                             + [bias_all[sl]]).astype(np.float32).reshape(1, -1)
        assert pk8.shape[1] == Lh["_n8"] and pk16.shape[1] == Lh["_n16"] \
            and pki.shape[1] == Lh["_ni"] and pkf.shape[1] == Lh["_nf"]
        in_maps.append(dict(pk8=pk8, pk16=pk16, pki=pki, pkf=pkf))

    _t0 = _time.perf_counter()
    try:
        res = bass_utils.run_bass_kernel_spmd(nc, in_maps,
                                              core_ids=list(range(N_CORES)))
    except Exception:
        # transient NRT/device fault: back off once and retry
        _time.sleep(10)
        _t0 = _time.perf_counter()
        res = bass_utils.run_bass_kernel_spmd(nc, in_maps,
                                              core_ids=list(range(N_CORES)))
    EXEC_SECONDS.append(_time.perf_counter() - _t0)
    out = np.concatenate([res.results[c]["out"][0] for c in range(N_CORES)])
    return out.astype(np.float32)
